# revision 13
# baseline (speedup 1.0000x reference)
"""Trainium2 Bass kernel for the 4-layer QRNN model.

Model (per reference):
    h0 = x @ fc1_w.T + fc1_b                       (B, S, 512) -> (B, S, 256)
    per layer l in 0..3 (time-major):
        y = W_l @ h + b_l                          (3H x H) gates
        z = tanh(y_z); f = sigmoid(y_f); o = sigmoid(y_o)
        c_t = f_t * c_{t-1} + (1 - f_t) * z_t      (sequential scan over S)
        h <- o * c ;  h_n[l] = c_last
    logits = h[S-1] @ fc_w.T + fc_b; class_hat = softmax(logits)
    returns (h_layer4, class_hat, h_n)

Distribution: data-parallel over batch B=64 -> 8 NeuronCores x 8 batch rows.
Params replicated. On-device layout is feature-major [h_partition, time] so
the recurrence maps to the DVE tensor_tensor_scan instruction
(state = f*state - (f-1)*z) along the free dimension; all matmuls contract
over h on partitions. The host marshals x/outputs between token-major and
feature-major as part of shard/unshard so every device DMA is contiguous.
"""

import numpy as np

# ---------------------------------------------------------------- config ----
P = 128          # partitions
S = 2048         # sequence length
KIN = 512        # input feature dim
H = 256          # hidden
G = 3 * H        # gates (z|f|o)
L = 4            # layers
B_FULL = 64      # total batch
NCORES = 8
BS = B_FULL // NCORES   # batch rows per core
NCLS = 10

KO = KIN // P    # 4  k-tiles for fc1
HO = H // P      # 2  h-tiles
MO = G // P      # 6  gate m-tiles
CH = 512         # fc1 matmul N-chunk (one PSUM bank)
LC = 1024        # layer chunk (gate psum tile free size, 2 banks)
NMM = 512        # matmul moving free dim (<= one PSUM bank of fp32)
NLC = S // LC    # scan chunks per (b, layer)

MM_BF16 = True   # bf16 matmul operands (1 cyc/row); False -> full fp32 (4 cyc/row)


# ------------------------------------------------------------ bass program --
def build_program(bs=BS, s=S, mm_bf16=MM_BF16):
    import concourse.bass as bass
    import concourse.bacc as bacc
    import concourse.mybir as mybir
    import concourse.tile as tile
    from contextlib import ExitStack

    f32 = mybir.dt.float32
    mmdt = mybir.dt.bfloat16 if mm_bf16 else f32
    Act = mybir.ActivationFunctionType
    Alu = mybir.AluOpType
    Ax = mybir.AxisListType

    nlc = s // LC
    nch = s // CH

    nc = bacc.Bacc()

    # inputs (host-marshaled layouts; see kernel() below)
    xT = nc.declare_dram_parameter("xT", [bs, P, KO, s], mmdt, False)          # x[b,s,k] -> [b,p,ko,s]
    fc1_wT = nc.declare_dram_parameter("fc1_wT", [P, KO, H], mmdt, False)      # fc1_w.T tiles
    qrnn_wT = nc.declare_dram_parameter("qrnn_wT", [P, L, HO, G], mmdt, False)
    qrnn_bT = nc.declare_dram_parameter("qrnn_bT", [P, L, MO], f32, False)
    fc_wT = nc.declare_dram_parameter("fc_wT", [P, HO, NCLS], f32, False)
    fc_b2 = nc.declare_dram_parameter("fc_b2", [NCLS, bs], f32, False)

    # outputs (feature-major; host transposes back)
    h_out = nc.declare_dram_parameter("h_out", [bs, P, HO, s], f32, isOutput=True)
    class_hat = nc.declare_dram_parameter("class_hat", [bs, NCLS], f32, isOutput=True)
    h_n_out = nc.declare_dram_parameter("h_n_out", [P, L, HO, bs], f32, isOutput=True)

    with ExitStack() as ctx:
        tc = ctx.enter_context(tile.TileContext(nc))
        wpool = ctx.enter_context(tc.tile_pool(name="wpool", bufs=1))
        xpool = ctx.enter_context(tc.tile_pool(name="xpool", bufs=3))
        hpool = ctx.enter_context(tc.tile_pool(name="hpool", bufs=3))
        gpool = ctx.enter_context(tc.tile_pool(name="gpool", bufs=3))
        cpool = ctx.enter_context(tc.tile_pool(name="cpool", bufs=3))
        spool = ctx.enter_context(tc.tile_pool(name="spool", bufs=2))
        fpsum = ctx.enter_context(tc.tile_pool(name="fpsum", bufs=2, space="PSUM"))
        gpsum = ctx.enter_context(tc.tile_pool(name="gpsum", bufs=3, space="PSUM"))

        # ---- load replicated params (all DMAs contiguous) ----
        fc1_wT_sb = wpool.tile([P, KO, H], mmdt, name="fc1_wT_sb")
        nc.sync.dma_start(out=fc1_wT_sb[:, :, :], in_=fc1_wT[:, :, :])
        qw_sb = wpool.tile([P, L, HO, G], mmdt, name="qw_sb")
        nc.sync.dma_start(out=qw_sb[:, :, :, :], in_=qrnn_wT[:, :, :, :])
        qb_sb = wpool.tile([P, L, MO], f32, name="qb_sb")
        nc.sync.dma_start(out=qb_sb[:, :, :], in_=qrnn_bT[:, :, :])
        fc_wT_sb = wpool.tile([P, HO, NCLS], f32, name="fc_wT_sb")
        nc.sync.dma_start(out=fc_wT_sb[:, :, :], in_=fc_wT[:, :, :])
        fc_b_sb = wpool.tile([NCLS, bs], f32, name="fc_b_sb")
        nc.sync.dma_start(out=fc_b_sb[:, :], in_=fc_b2[:, :])

        hn_stage = wpool.tile([P, L, HO, bs], f32, name="hn_stage")
        lg_stage = wpool.tile([P, HO, bs], f32, name="lg_stage")

        for b in range(bs):
            # ---- x load (feature-major tiles) ----
            xc = xpool.tile([P, KO, s], mmdt, tag="xc", name=f"xc_b{b}")
            nc.sync.dma_start(out=xc[:, :, :], in_=xT[b])

            # ---- fc1: h0[h, s] = fc1_w @ x_b^T + fc1_b ----
            h0 = hpool.tile([P, HO, s], mmdt, tag="h", name=f"h0_b{b}")
            for cc in range(nch):
                csl = slice(cc * CH, (cc + 1) * CH)
                for ho in range(HO):
                    ps = fpsum.tile([P, CH], f32, tag="fps", name=f"fc1ps_b{b}_c{cc}_h{ho}")
                    for ko in range(KO):
                        nc.tensor.matmul(
                            ps[:, :],
                            fc1_wT_sb[:, ko, ho * P:(ho + 1) * P],
                            xc[:, ko, csl],
                            start=(ko == 0),
                            stop=(ko == KO - 1),
                        )
                    nc.vector.tensor_copy(h0[:, ho, csl], ps[:, :])

            # ---- QRNN layers ----
            hprev = h0
            for l in range(L):
                if l < L - 1:
                    hnext = hpool.tile([P, HO, s], mmdt, tag="h", name=f"h_b{b}_l{l}")
                else:
                    hnext = hpool.tile([P, HO, s], f32, tag="hf", bufs=2, name=f"h_b{b}_l{l}")
                c_prev = [None, None]
                for cc in range(nlc):
                    lsl = slice(cc * LC, (cc + 1) * LC)
                    ytiles = []
                    for m in range(MO):
                        yp = gpsum.tile([P, LC], f32, tag="yps", name=f"yp_b{b}_l{l}_c{cc}_m{m}")
                        for nh in range(LC // NMM):
                            nsl = slice(nh * NMM, (nh + 1) * NMM)
                            gsl = slice(cc * LC + nh * NMM, cc * LC + (nh + 1) * NMM)
                            for ho in range(HO):
                                nc.tensor.matmul(
                                    yp[:, nsl],
                                    qw_sb[:, l, ho, m * P:(m + 1) * P],
                                    hprev[:, ho, gsl],
                                    start=(ho == 0),
                                    stop=(ho == HO - 1),
                                )
                        ytiles.append(yp)
                    # gates: m 0..1 -> z (tanh), 2..3 -> f (sigmoid), 4..5 -> o
                    zt, ft, ot = [None, None], [None, None], [None, None]
                    for m in range(MO):
                        kind, ho = divmod(m, HO)
                        func = Act.Tanh if kind == 0 else Act.Sigmoid
                        gname = "zfo"[kind]
                        gt = gpool.tile([P, LC], f32, tag=gname,
                                        name=f"{gname}_b{b}_l{l}_c{cc}_h{ho}")
                        nc.scalar.activation(
                            out=gt[:, :], in_=ytiles[m][:, :], func=func,
                            bias=qb_sb[:, l, m:m + 1], scale=1.0,
                        )
                        [zt, ft, ot][kind][ho] = gt
                    for ho in range(HO):
                        fz = gpool.tile([P, LC], f32, tag="fz", name=f"fz_b{b}_l{l}_c{cc}_h{ho}")
                        # fz = (f - 1) * z
                        nc.vector.scalar_tensor_tensor(
                            out=fz[:, :], in0=ft[ho][:, :], scalar=1.0, in1=zt[ho][:, :],
                            op0=Alu.subtract, op1=Alu.mult,
                        )
                        cch = cpool.tile([P, LC], f32, tag="c", name=f"c_b{b}_l{l}_c{cc}_h{ho}")
                        init = 0.0 if cc == 0 else c_prev[ho][:, LC - 1:LC]
                        # c_t = f_t * c_{t-1} - (f_t - 1) * z_t
                        nc.vector.tensor_tensor_scan(
                            out=cch[:, :], data0=ft[ho][:, :], data1=fz[:, :],
                            initial=init, op0=Alu.mult, op1=Alu.subtract,
                        )
                        # h_next = o * c   (on gpsimd to unload the vector engine)
                        nc.vector.tensor_mul(hnext[:, ho, lsl], ot[ho][:, :], cch[:, :])
                        c_prev[ho] = cch
                        if cc == nlc - 1:
                            nc.vector.tensor_copy(hn_stage[:, l, ho, b:b + 1],
                                                  cch[:, LC - 1:LC])
                hprev = hnext

            # ---- outputs for this b ----
            nc.sync.dma_start(out=h_out[b], in_=hprev[:, :, :])
            for ho in range(HO):
                nc.vector.tensor_copy(lg_stage[:, ho, b:b + 1], hprev[:, ho, s - 1:s])

        # ---- h_n ----
        nc.sync.dma_start(out=h_n_out[:, :, :, :], in_=hn_stage[:, :, :, :])

        # ---- logits + softmax ----
        lp = fpsum.tile([NCLS, bs], f32, tag="fps", name="logits_ps")
        for ho in range(HO):
            nc.tensor.matmul(
                lp[:, :], fc_wT_sb[:, ho, :], lg_stage[:, ho, :],
                start=(ho == 0), stop=(ho == HO - 1),
            )
        lsb = spool.tile([NCLS, bs], f32, name="lsb")
        nc.vector.tensor_add(lsb[:, :], lp[:, :], fc_b_sb[:, :])
        # transpose [NCLS, bs] -> [bs, NCLS] via DVE 32x32 block transpose
        ltp = spool.tile([32, 32], f32, name="ltp")
        nc.vector.memset(ltp[:, :], 0.0)
        nc.vector.tensor_copy(ltp[:NCLS, :bs], lsb[:, :])
        ltt = spool.tile([32, 32], f32, name="ltt")
        nc.vector.transpose(ltt[:, :], ltp[:, :])
        nmx = spool.tile([bs, 1], f32, name="nmx")
        nc.vector.tensor_reduce(out=nmx[:, :], in_=ltt[:bs, :NCLS], axis=Ax.X,
                                op=Alu.max, negate=True)
        ex = spool.tile([bs, NCLS], f32, name="ex")
        nc.scalar.activation(out=ex[:, :], in_=ltt[:bs, :NCLS], func=Act.Exp,
                             bias=nmx[:, 0:1], scale=1.0)
        sm = spool.tile([bs, 1], f32, name="sm")
        nc.vector.tensor_reduce(out=sm[:, :], in_=ex[:, :], axis=Ax.X, op=Alu.add)
        rc = spool.tile([bs, 1], f32, name="rc")
        nc.vector.reciprocal(rc[:, :], sm[:, :])
        co = spool.tile([bs, NCLS], f32, name="co")
        nc.scalar.activation(out=co[:, :], in_=ex[:, :], func=Act.Copy,
                             scale=rc[:, 0:1])
        nc.sync.dma_start(out=class_hat[:, :], in_=co[:, :])

    nc.compile()
    return nc


# ----------------------------------------------------------- host wrapper ---
def marshal_params(fc1_w, fc1_b, qrnn_w, qrnn_b, fc_w, fc_b, bs=BS):
    fc1_w = np.asarray(fc1_w, np.float32)
    fc1_b = np.asarray(fc1_b, np.float32)
    qrnn_w = np.asarray(qrnn_w, np.float32)
    qrnn_b = np.asarray(qrnn_b, np.float32)
    fc_w = np.asarray(fc_w, np.float32)
    fc_b = np.asarray(fc_b, np.float32)
    import ml_dtypes
    bf16 = ml_dtypes.bfloat16
    # fold fc1_b into layer-0 gate bias: y0 = W0(x@fc1_w.T) + (W0@fc1_b + b0)
    qrnn_b_eff = qrnn_b.copy()
    qrnn_b_eff[0] = qrnn_b[0] + qrnn_w[0] @ fc1_b
    return {
        "fc1_wT": np.ascontiguousarray(fc1_w.reshape(H, KO, P).transpose(2, 1, 0)).astype(bf16),
        "qrnn_wT": np.ascontiguousarray(qrnn_w.reshape(L, G, HO, P).transpose(3, 0, 2, 1)).astype(bf16),
        "qrnn_bT": np.ascontiguousarray(qrnn_b_eff.reshape(L, MO, P).transpose(2, 0, 1)),
        "fc_wT": np.ascontiguousarray(fc_w.reshape(NCLS, HO, P).transpose(2, 1, 0)),
        "fc_b2": np.ascontiguousarray(np.tile(fc_b.reshape(NCLS, 1), (1, bs))),
    }


def marshal_x_shard(x_shard):
    # (bs, S, KIN) -> (bs, P, KO, S), bf16 matmul operand
    import ml_dtypes
    bs, s, _ = x_shard.shape
    return np.ascontiguousarray(
        np.asarray(x_shard, np.float32).reshape(bs, s, KO, P).transpose(0, 3, 2, 1)
    ).astype(ml_dtypes.bfloat16)


def unmarshal_outputs(results):
    # concat per-core outputs back to full tensors
    h_parts, ch_parts, hn_parts = [], [], []
    for r in results:
        ho = r["h_out"]                       # (bs, P, HO, S)
        bs = ho.shape[0]
        h_parts.append(ho.transpose(3, 0, 2, 1).reshape(S, bs, H))
        ch_parts.append(r["class_hat"])
        hn = r["h_n_out"]                     # (P, L, HO, bs)
        hn_parts.append(hn.transpose(1, 3, 2, 0).reshape(L, bs, H))
    h = np.ascontiguousarray(np.concatenate(h_parts, axis=1))
    class_hat = np.ascontiguousarray(np.concatenate(ch_parts, axis=0))
    h_n = np.ascontiguousarray(np.concatenate(hn_parts, axis=1))
    return h, class_hat, h_n


_PROGRAM_CACHE = {}


def _get_program():
    if "nc" not in _PROGRAM_CACHE:
        _PROGRAM_CACHE["nc"] = build_program()
    return _PROGRAM_CACHE["nc"]


def run_device(inputs, trace=False, trace_kwargs=None):
    """Run the SPMD kernel on 8 cores. Returns (outputs_tuple, BassKernelResults)."""
    from concourse.bass_utils import run_bass_kernel_spmd

    x = np.asarray(inputs["x"], np.float32)
    params = marshal_params(
        inputs["fc1_w"], inputs["fc1_b"], inputs["qrnn_w"],
        inputs["qrnn_b"], inputs["fc_w"], inputs["fc_b"],
    )
    in_maps = []
    for c in range(NCORES):
        m = dict(params)
        m["xT"] = marshal_x_shard(x[c * BS:(c + 1) * BS])
        in_maps.append(m)

    nc = _get_program()
    kw = {}
    if trace:
        kw["trace"] = True
        if trace_kwargs:
            kw.update(trace_kwargs)
    res = run_bass_kernel_spmd(nc, in_maps, core_ids=list(range(NCORES)), **kw)
    return unmarshal_outputs(res.results), res


def kernel(x, fc1_w, fc1_b, qrnn_w, qrnn_b, fc_w, fc_b):
    (h, class_hat, h_n), _ = run_device(dict(
        x=x, fc1_w=fc1_w, fc1_b=fc1_b, qrnn_w=qrnn_w, qrnn_b=qrnn_b,
        fc_w=fc_w, fc_b=fc_b,
    ))
    return h, class_hat, h_n


# revision 14
# speedup vs baseline: 1.1457x; 1.1457x over previous
"""Trainium2 Bass kernel for the 4-layer QRNN model.

Model (per reference):
    h0 = x @ fc1_w.T + fc1_b                       (B, S, 512) -> (B, S, 256)
    per layer l in 0..3 (time-major):
        y = W_l @ h + b_l                          (3H x H) gates
        z = tanh(y_z); f = sigmoid(y_f); o = sigmoid(y_o)
        c_t = f_t * c_{t-1} + (1 - f_t) * z_t      (sequential scan over S)
        h <- o * c ;  h_n[l] = c_last
    logits = h[S-1] @ fc_w.T + fc_b; class_hat = softmax(logits)
    returns (h_layer4, class_hat, h_n)

Distribution: data-parallel over batch B=64 -> 8 NeuronCores x 8 batch rows.
Params replicated. On-device layout is feature-major [h_partition, time] so
the recurrence maps to the DVE tensor_tensor_scan instruction
(state = f*state - (f-1)*z) along the free dimension; all matmuls contract
over h on partitions. The host marshals x/outputs between token-major and
feature-major as part of shard/unshard so every device DMA is contiguous.
"""

import numpy as np

# ---------------------------------------------------------------- config ----
P = 128          # partitions
S = 2048         # sequence length
KIN = 512        # input feature dim
H = 256          # hidden
G = 3 * H        # gates (z|f|o)
L = 4            # layers
B_FULL = 64      # total batch
NCORES = 8
BS = B_FULL // NCORES   # batch rows per core
NCLS = 10

KO = KIN // P    # 4  k-tiles for fc1
HO = H // P      # 2  h-tiles
MO = G // P      # 6  gate m-tiles
CH = 512         # fc1 matmul N-chunk (one PSUM bank)
LC = 1024        # layer chunk (gate psum tile free size, 2 banks)
NMM = 512        # matmul moving free dim (<= one PSUM bank of fp32)
NLC = S // LC    # scan chunks per (b, layer)

MM_BF16 = True   # bf16 matmul operands (1 cyc/row); False -> full fp32 (4 cyc/row)


# ------------------------------------------------------------ bass program --
def build_program(bs=BS, s=S, mm_bf16=MM_BF16):
    import concourse.bass as bass
    import concourse.bacc as bacc
    import concourse.mybir as mybir
    import concourse.tile as tile
    from contextlib import ExitStack

    f32 = mybir.dt.float32
    mmdt = mybir.dt.bfloat16 if mm_bf16 else f32
    Act = mybir.ActivationFunctionType
    Alu = mybir.AluOpType
    Ax = mybir.AxisListType

    nlc = s // LC
    nch = s // CH

    nc = bacc.Bacc()

    # inputs (host-marshaled layouts; see kernel() below)
    xT = nc.declare_dram_parameter("xT", [bs, P, KO, s], mmdt, False)          # x[b,s,k] -> [b,p,ko,s]
    fc1_wT = nc.declare_dram_parameter("fc1_wT", [P, KO, H], mmdt, False)      # fc1_w.T tiles
    qrnn_wT = nc.declare_dram_parameter("qrnn_wT", [P, L, HO, G], mmdt, False)
    qrnn_bT = nc.declare_dram_parameter("qrnn_bT", [P, L, MO], f32, False)
    fc_wT = nc.declare_dram_parameter("fc_wT", [P, HO, NCLS], f32, False)
    fc_b2 = nc.declare_dram_parameter("fc_b2", [NCLS, bs], f32, False)

    # outputs (feature-major; host transposes back)
    h_out = nc.declare_dram_parameter("h_out", [bs, P, HO, s], f32, isOutput=True)
    class_hat = nc.declare_dram_parameter("class_hat", [bs, NCLS], f32, isOutput=True)
    h_n_out = nc.declare_dram_parameter("h_n_out", [P, L, HO, bs], f32, isOutput=True)

    with ExitStack() as ctx:
        tc = ctx.enter_context(tile.TileContext(nc))
        wpool = ctx.enter_context(tc.tile_pool(name="wpool", bufs=1))
        xpool = ctx.enter_context(tc.tile_pool(name="xpool", bufs=3))
        hpool = ctx.enter_context(tc.tile_pool(name="hpool", bufs=3))
        gpool = ctx.enter_context(tc.tile_pool(name="gpool", bufs=3))
        cpool = ctx.enter_context(tc.tile_pool(name="cpool", bufs=3))
        spool = ctx.enter_context(tc.tile_pool(name="spool", bufs=2))
        fpsum = ctx.enter_context(tc.tile_pool(name="fpsum", bufs=2, space="PSUM"))
        gpsum = ctx.enter_context(tc.tile_pool(name="gpsum", bufs=3, space="PSUM"))

        # ---- load replicated params (all DMAs contiguous) ----
        fc1_wT_sb = wpool.tile([P, KO, H], mmdt, name="fc1_wT_sb")
        nc.sync.dma_start(out=fc1_wT_sb[:, :, :], in_=fc1_wT[:, :, :])
        qw_sb = wpool.tile([P, L, HO, G], mmdt, name="qw_sb")
        nc.sync.dma_start(out=qw_sb[:, :, :, :], in_=qrnn_wT[:, :, :, :])
        qb_sb = wpool.tile([P, L, MO], f32, name="qb_sb")
        nc.sync.dma_start(out=qb_sb[:, :, :], in_=qrnn_bT[:, :, :])
        fc_wT_sb = wpool.tile([P, HO, NCLS], f32, name="fc_wT_sb")
        nc.sync.dma_start(out=fc_wT_sb[:, :, :], in_=fc_wT[:, :, :])
        fc_b_sb = wpool.tile([NCLS, bs], f32, name="fc_b_sb")
        nc.sync.dma_start(out=fc_b_sb[:, :], in_=fc_b2[:, :])

        hn_stage = wpool.tile([P, L, HO, bs], f32, name="hn_stage")
        lg_stage = wpool.tile([P, HO, bs], f32, name="lg_stage")

        for b in range(bs):
            # ---- x load (feature-major tiles) ----
            xc = xpool.tile([P, KO, s], mmdt, tag="xc", name=f"xc_b{b}")
            nc.sync.dma_start(out=xc[:, :, :], in_=xT[b])

            # ---- fc1: h0[h, s] = fc1_w @ x_b^T + fc1_b ----
            h0 = hpool.tile([P, HO, s], mmdt, tag="h", name=f"h0_b{b}")
            for cc in range(nch):
                csl = slice(cc * CH, (cc + 1) * CH)
                for ho in range(HO):
                    ps = fpsum.tile([P, CH], f32, tag="fps", name=f"fc1ps_b{b}_c{cc}_h{ho}")
                    for ko in range(KO):
                        nc.tensor.matmul(
                            ps[:, :],
                            fc1_wT_sb[:, ko, ho * P:(ho + 1) * P],
                            xc[:, ko, csl],
                            start=(ko == 0),
                            stop=(ko == KO - 1),
                        )
                    nc.vector.tensor_copy(h0[:, ho, csl], ps[:, :])

            # ---- QRNN layers ----
            hprev = h0
            for l in range(L):
                # gates/scan in bf16 for layers 0..2 (DVE 2x packing);
                # layer 3 stays fp32 for output accuracy
                gdt = mmdt if l < L - 1 else f32
                if l < L - 1:
                    hnext = hpool.tile([P, HO, s], mmdt, tag="h", name=f"h_b{b}_l{l}")
                else:
                    hnext = hpool.tile([P, HO, s], f32, tag="hf", bufs=2, name=f"h_b{b}_l{l}")
                c_prev = [None, None]
                for cc in range(nlc):
                    lsl = slice(cc * LC, (cc + 1) * LC)
                    ytiles = []
                    for m in range(MO):
                        yp = gpsum.tile([P, LC], f32, tag="yps", name=f"yp_b{b}_l{l}_c{cc}_m{m}")
                        for nh in range(LC // NMM):
                            nsl = slice(nh * NMM, (nh + 1) * NMM)
                            gsl = slice(cc * LC + nh * NMM, cc * LC + (nh + 1) * NMM)
                            for ho in range(HO):
                                nc.tensor.matmul(
                                    yp[:, nsl],
                                    qw_sb[:, l, ho, m * P:(m + 1) * P],
                                    hprev[:, ho, gsl],
                                    start=(ho == 0),
                                    stop=(ho == HO - 1),
                                )
                        ytiles.append(yp)
                    # gates: m 0..1 -> z (tanh), 2..3 -> f (sigmoid), 4..5 -> o
                    zt, ft, ot = [None, None], [None, None], [None, None]
                    for m in range(MO):
                        kind, ho = divmod(m, HO)
                        func = Act.Tanh if kind == 0 else Act.Sigmoid
                        gname = "zfo"[kind]
                        gt = gpool.tile([P, LC], gdt, tag=gname,
                                        name=f"{gname}_b{b}_l{l}_c{cc}_h{ho}")
                        nc.scalar.activation(
                            out=gt[:, :], in_=ytiles[m][:, :], func=func,
                            bias=qb_sb[:, l, m:m + 1], scale=1.0,
                        )
                        [zt, ft, ot][kind][ho] = gt
                    for ho in range(HO):
                        fz = gpool.tile([P, LC], gdt, tag="fz", name=f"fz_b{b}_l{l}_c{cc}_h{ho}")
                        # fz = (f - 1) * z
                        nc.vector.scalar_tensor_tensor(
                            out=fz[:, :], in0=ft[ho][:, :], scalar=1.0, in1=zt[ho][:, :],
                            op0=Alu.subtract, op1=Alu.mult,
                        )
                        cch = cpool.tile([P, LC], gdt, tag="c", name=f"c_b{b}_l{l}_c{cc}_h{ho}")
                        init = 0.0 if cc == 0 else c_prev[ho][:, LC - 1:LC]
                        # c_t = f_t * c_{t-1} - (f_t - 1) * z_t
                        nc.vector.tensor_tensor_scan(
                            out=cch[:, :], data0=ft[ho][:, :], data1=fz[:, :],
                            initial=init, op0=Alu.mult, op1=Alu.subtract,
                        )
                        nc.vector.tensor_mul(hnext[:, ho, lsl], ot[ho][:, :], cch[:, :])
                        c_prev[ho] = cch
                        if cc == nlc - 1:
                            nc.vector.tensor_copy(hn_stage[:, l, ho, b:b + 1],
                                                  cch[:, LC - 1:LC])
                hprev = hnext

            # ---- outputs for this b ----
            nc.sync.dma_start(out=h_out[b], in_=hprev[:, :, :])
            for ho in range(HO):
                nc.vector.tensor_copy(lg_stage[:, ho, b:b + 1], hprev[:, ho, s - 1:s])

        # ---- h_n ----
        nc.sync.dma_start(out=h_n_out[:, :, :, :], in_=hn_stage[:, :, :, :])

        # ---- logits + softmax ----
        lp = fpsum.tile([NCLS, bs], f32, tag="fps", name="logits_ps")
        for ho in range(HO):
            nc.tensor.matmul(
                lp[:, :], fc_wT_sb[:, ho, :], lg_stage[:, ho, :],
                start=(ho == 0), stop=(ho == HO - 1),
            )
        lsb = spool.tile([NCLS, bs], f32, name="lsb")
        nc.vector.tensor_add(lsb[:, :], lp[:, :], fc_b_sb[:, :])
        # transpose [NCLS, bs] -> [bs, NCLS] via DVE 32x32 block transpose
        ltp = spool.tile([32, 32], f32, name="ltp")
        nc.vector.memset(ltp[:, :], 0.0)
        nc.vector.tensor_copy(ltp[:NCLS, :bs], lsb[:, :])
        ltt = spool.tile([32, 32], f32, name="ltt")
        nc.vector.transpose(ltt[:, :], ltp[:, :])
        nmx = spool.tile([bs, 1], f32, name="nmx")
        nc.vector.tensor_reduce(out=nmx[:, :], in_=ltt[:bs, :NCLS], axis=Ax.X,
                                op=Alu.max, negate=True)
        ex = spool.tile([bs, NCLS], f32, name="ex")
        nc.scalar.activation(out=ex[:, :], in_=ltt[:bs, :NCLS], func=Act.Exp,
                             bias=nmx[:, 0:1], scale=1.0)
        sm = spool.tile([bs, 1], f32, name="sm")
        nc.vector.tensor_reduce(out=sm[:, :], in_=ex[:, :], axis=Ax.X, op=Alu.add)
        rc = spool.tile([bs, 1], f32, name="rc")
        nc.vector.reciprocal(rc[:, :], sm[:, :])
        co = spool.tile([bs, NCLS], f32, name="co")
        nc.scalar.activation(out=co[:, :], in_=ex[:, :], func=Act.Copy,
                             scale=rc[:, 0:1])
        nc.sync.dma_start(out=class_hat[:, :], in_=co[:, :])

    nc.compile()
    return nc


# ----------------------------------------------------------- host wrapper ---
def marshal_params(fc1_w, fc1_b, qrnn_w, qrnn_b, fc_w, fc_b, bs=BS):
    fc1_w = np.asarray(fc1_w, np.float32)
    fc1_b = np.asarray(fc1_b, np.float32)
    qrnn_w = np.asarray(qrnn_w, np.float32)
    qrnn_b = np.asarray(qrnn_b, np.float32)
    fc_w = np.asarray(fc_w, np.float32)
    fc_b = np.asarray(fc_b, np.float32)
    import ml_dtypes
    bf16 = ml_dtypes.bfloat16
    # fold fc1_b into layer-0 gate bias: y0 = W0(x@fc1_w.T) + (W0@fc1_b + b0)
    qrnn_b_eff = qrnn_b.copy()
    qrnn_b_eff[0] = qrnn_b[0] + qrnn_w[0] @ fc1_b
    return {
        "fc1_wT": np.ascontiguousarray(fc1_w.reshape(H, KO, P).transpose(2, 1, 0)).astype(bf16),
        "qrnn_wT": np.ascontiguousarray(qrnn_w.reshape(L, G, HO, P).transpose(3, 0, 2, 1)).astype(bf16),
        "qrnn_bT": np.ascontiguousarray(qrnn_b_eff.reshape(L, MO, P).transpose(2, 0, 1)),
        "fc_wT": np.ascontiguousarray(fc_w.reshape(NCLS, HO, P).transpose(2, 1, 0)),
        "fc_b2": np.ascontiguousarray(np.tile(fc_b.reshape(NCLS, 1), (1, bs))),
    }


def marshal_x_shard(x_shard):
    # (bs, S, KIN) -> (bs, P, KO, S), bf16 matmul operand
    import ml_dtypes
    bs, s, _ = x_shard.shape
    return np.ascontiguousarray(
        np.asarray(x_shard, np.float32).reshape(bs, s, KO, P).transpose(0, 3, 2, 1)
    ).astype(ml_dtypes.bfloat16)


def unmarshal_outputs(results):
    # concat per-core outputs back to full tensors
    h_parts, ch_parts, hn_parts = [], [], []
    for r in results:
        ho = r["h_out"]                       # (bs, P, HO, S)
        bs = ho.shape[0]
        h_parts.append(ho.transpose(3, 0, 2, 1).reshape(S, bs, H))
        ch_parts.append(r["class_hat"])
        hn = r["h_n_out"]                     # (P, L, HO, bs)
        hn_parts.append(hn.transpose(1, 3, 2, 0).reshape(L, bs, H))
    h = np.ascontiguousarray(np.concatenate(h_parts, axis=1))
    class_hat = np.ascontiguousarray(np.concatenate(ch_parts, axis=0))
    h_n = np.ascontiguousarray(np.concatenate(hn_parts, axis=1))
    return h, class_hat, h_n


_PROGRAM_CACHE = {}


def _get_program():
    if "nc" not in _PROGRAM_CACHE:
        _PROGRAM_CACHE["nc"] = build_program()
    return _PROGRAM_CACHE["nc"]


def run_device(inputs, trace=False, trace_kwargs=None):
    """Run the SPMD kernel on 8 cores. Returns (outputs_tuple, BassKernelResults)."""
    from concourse.bass_utils import run_bass_kernel_spmd

    x = np.asarray(inputs["x"], np.float32)
    params = marshal_params(
        inputs["fc1_w"], inputs["fc1_b"], inputs["qrnn_w"],
        inputs["qrnn_b"], inputs["fc_w"], inputs["fc_b"],
    )
    in_maps = []
    for c in range(NCORES):
        m = dict(params)
        m["xT"] = marshal_x_shard(x[c * BS:(c + 1) * BS])
        in_maps.append(m)

    nc = _get_program()
    kw = {}
    if trace:
        kw["trace"] = True
        if trace_kwargs:
            kw.update(trace_kwargs)
    res = run_bass_kernel_spmd(nc, in_maps, core_ids=list(range(NCORES)), **kw)
    return unmarshal_outputs(res.results), res


def kernel(x, fc1_w, fc1_b, qrnn_w, qrnn_b, fc_w, fc_b):
    (h, class_hat, h_n), _ = run_device(dict(
        x=x, fc1_w=fc1_w, fc1_b=fc1_b, qrnn_w=qrnn_w, qrnn_b=qrnn_b,
        fc_w=fc_w, fc_b=fc_b,
    ))
    return h, class_hat, h_n
